# revision 5
# baseline (speedup 1.0000x reference)
"""GIN message-passing kernel for Trainium2, 8 NeuronCores.

Strategy:
  - Nodes sharded 12500/core (by original id range); within a core, nodes are
    ordered by node_id so the 200k-row embedding table can be gathered with
    int16 indices relative to 32768-row chunks.
  - Per GIN layer: y = x @ W is computed locally per 128-node tile, the y
    blocks are AllGathered (f32), then each core aggregates its in-edges with
    dma_gather (512B rows) + a one-hot "S matrix" PSUM matmul per 128-edge
    chunk (order-agnostic, collision-free segment sum).
  - Graph mean-pool is a one-hot matmul accumulated into an SBUF [128,4*129]
    buffer (col 128 of each graph-chunk = counts), AllReduced across cores;
    the tiny MLP head runs replicated on every core.

All dtypes f32 on device. Host preprocessing builds int16 index tables,
one-hot selector columns, and uniform (SPMD) slot layouts padded to the max
across cores.
"""
import sys

sys.path.insert(0, "/opt/trn_rl_repo")

import numpy as np

P = 128
N_NODES = 100000
HID = 128
N_GRAPHS = 512
NODE_VOCAB = 200000
EMB_CHUNK = 32768
NCORES = 8
NPC = N_NODES // NCORES  # nodes per core
BN_EPS = 1e-5
GROUP_T = 4  # dst tiles per gather group (PSUM agg tiles alive at once)


def _round_up(x, m):
    return (int(x) + m - 1) // m * m


def _apply_tile_patch():
    """walrus in this container rejects the TileContext final drain when it
    carries >1 sem wait; split the waits across extra drain instructions."""
    import concourse.tile as tilemod
    from concourse.vector_clock import ScopedClock

    def _drain_and_barrier_split(self, tick_clock, wait_clock):
        drain_inst = self.nc.sync.drain()
        wait_clock.add_sem_waits(
            drain_inst.ins, ScopedClock({None: tick_clock.global_clock})
        )
        si = drain_inst.ins.sync_info
        if si is not None and si.on_wait and len(si.on_wait) > 1:
            waits = list(si.on_wait)
            si.on_wait = waits[:1]
            for w in waits[1:]:
                extra = self.nc.sync.drain()
                esi = extra.ins.sync_info
                if esi is None:
                    import concourse.mybir as mybir

                    extra.ins.sync_info = mybir.SyncInfo(on_wait=[w], on_update=[])
                else:
                    esi.on_wait = list(esi.on_wait) + [w]

        self.nc.all_engine_barrier()
        assert self.sems is not None
        popped = self.nc._tile_sem_poison_stack.pop()
        assert popped is self._sem_poison
        self.nc.clear_and_free_semaphores(list(self.sems.allocated().values()))
        self.nc.all_engine_barrier()

    tilemod.TileContext._drain_and_barrier = _drain_and_barrier_split


def _wrap_idx_into(dst, col0, idx):
    """Place idx list into dst[:, col0:col0+len/16] in the 16-partition wrapped
    layout dma_gather expects, replicated across the 8 gpsimd cores."""
    n = idx.shape[0]
    ncols = n // 16
    t = idx.reshape(ncols, 16).T.astype(np.int16)  # [16, ncols]
    dst[:, col0 : col0 + ncols] = np.tile(t, (8, 1))


class _Plan:
    """Uniform (SPMD) layout: chunk caps for embedding gathers and per
    (dst-tile, src-quarter) edge-segment caps, shared by all cores."""

    def __init__(self, node_id, edge_src, edge_dst):
        # ---- per-core node ordering (sorted by node_id) ----
        self.order = []  # per core: orig ids in slot order (real slots)
        self.chunk_counts = np.zeros((NCORES, 7), np.int64)
        for c in range(NCORES):
            ids = np.arange(c * NPC, (c + 1) * NPC)
            ids = ids[np.argsort(node_id[ids], kind="stable")]
            self.order.append(ids)
            self.chunk_counts[c] = np.bincount(node_id[ids] // EMB_CHUNK, minlength=7)
        self.emb_caps = np.array(
            [_round_up(self.chunk_counts[:, k].max(), P) for k in range(7)]
        )
        self.emb_offs = np.concatenate([[0], np.cumsum(self.emb_caps)])[:7]
        self.s_local = int(self.emb_caps.sum())
        self.nt = self.s_local // P  # dst tiles per core
        assert 2 * self.s_local < 32768, "int16 idx range exceeded"

        # ---- slot maps ----
        slot_of = np.full(N_NODES, -1, np.int64)  # local slot of a node
        for c in range(NCORES):
            ids = self.order[c]
            ch = node_id[ids] // EMB_CHUNK
            pos = np.zeros(len(ids), np.int64)
            for k in range(7):
                m = ch == k
                pos[m] = self.emb_offs[k] + np.arange(m.sum())
            slot_of[ids] = pos
        self.slot_of = slot_of
        self.gslot = (np.arange(N_NODES) // NPC) * self.s_local + slot_of

        # ---- edges (incl self loops), grouped per dst core ----
        src = np.concatenate([edge_src, np.arange(N_NODES)])
        dst = np.concatenate([edge_dst, np.arange(N_NODES)])
        dcore = dst // NPC
        self.core_edges = []  # per core: (tile, c4, rel_idx, dst_col)
        seg_counts = np.zeros((NCORES, self.nt, 4), np.int64)
        two_s = 2 * self.s_local
        for c in range(NCORES):
            m = dcore == c
            s, d = src[m], dst[m]
            delta = slot_of[d]
            t = delta // P
            sig = self.gslot[s]
            c4 = sig // two_s
            rel = sig - c4 * two_s
            self.core_edges.append((t, c4, rel, delta % P))
            np.add.at(seg_counts[c], (t, c4), 1)
        caps = seg_counts.max(axis=0)  # [nt, 4]
        self.seg_caps = ((caps + P - 1) // P) * P

        # ---- group layout: for g: for c4: for t in g: segment ----
        self.groups = [
            list(range(g, min(g + GROUP_T, self.nt)))
            for g in range(0, self.nt, GROUP_T)
        ]
        self.instr = {}  # (gi, c4) -> (slot_off, length)
        self.chunk_tile = []  # per chunk: tile index (traversal order)
        self.chunk_of_seg = {}  # (t, c4) -> first chunk id
        self.seg_off = {}  # (t, c4) -> slot offset
        off = 0
        for gi, g in enumerate(self.groups):
            for c4 in range(4):
                ln = int(sum(self.seg_caps[t, c4] for t in g))
                self.instr[(gi, c4)] = (off, ln)
                so = off
                for t in g:
                    self.chunk_of_seg[(t, c4)] = len(self.chunk_tile)
                    self.seg_off[(t, c4)] = so
                    for _ in range(int(self.seg_caps[t, c4]) // P):
                        self.chunk_tile.append(t)
                    so += int(self.seg_caps[t, c4])
                off += ln
        self.tot_slots = off
        self.nchunks = len(self.chunk_tile)

    def core_tables(self, c):
        """Build per-core idx (int16 wrapped) and dstslot (f32) tables."""
        t, c4, rel, dcol = self.core_edges[c]
        okey = t * 4 + c4
        o = np.argsort(okey, kind="stable")
        t, c4, rel, dcol = t[o], c4[o], rel[o], dcol[o]
        seg_key = t * 4 + c4
        uniq, start_idx, cnts = np.unique(
            seg_key, return_index=True, return_counts=True
        )
        flat_idx = np.zeros(self.tot_slots, np.int16)
        flat_dst = np.full(self.tot_slots, -1.0, np.float32)
        for u, si, cnt in zip(uniq, start_idx, cnts):
            tt, cc = int(u) // 4, int(u) % 4
            base = self.seg_off[(tt, cc)]
            flat_idx[base : base + cnt] = rel[si : si + cnt].astype(np.int16)
            flat_dst[base : base + cnt] = dcol[si : si + cnt].astype(np.float32)
        eidx = np.zeros((P, self.tot_slots // 16), np.int16)
        _wrap_idx_into(eidx, 0, flat_idx)
        dstslot = np.ascontiguousarray(
            flat_dst.reshape(self.nchunks, P).T
        )  # [128, nchunks]
        return eidx, dstslot


def _build(inputs):
    import concourse.mybir as mybir
    from concourse import bacc
    from concourse.tile import TileContext

    _apply_tile_patch()
    dt = mybir.dt
    f32 = np.float32

    z = np.asarray(inputs["z"]).astype(np.int64)
    node_id = np.asarray(inputs["node_id"]).astype(np.int64)
    batch = np.asarray(inputs["batch"]).astype(np.int64)
    ei = np.asarray(inputs["edge_index"]).astype(np.int64)

    plan = _Plan(node_id, ei[0], ei[1])
    S_LOCAL, NT = plan.s_local, plan.nt

    z_emb = np.asarray(inputs["z_emb"], f32)
    node_emb = np.asarray(inputs["node_emb"], f32)
    W0 = np.asarray(inputs["W0"], f32)
    Ws = np.stack(
        [W0[:HID], W0[HID:], np.asarray(inputs["W1"], f32), np.asarray(inputs["W2"], f32)]
    )
    brow = np.stack(
        [
            np.asarray(inputs["b0"], f32).reshape(1, P),
            np.asarray(inputs["b1"], f32).reshape(1, P),
            np.asarray(inputs["b2"], f32).reshape(1, P),
        ]
    )  # [3,1,128]
    mlp_W1 = np.asarray(inputs["mlp_W1"], f32)
    mlp_W2 = np.asarray(inputs["mlp_W2"], f32)  # [128, 2]
    gam = np.asarray(inputs["bn_gamma"], f32) / np.sqrt(1.0 + BN_EPS)
    tprime = np.asarray(inputs["mlp_b1"], f32) * gam + np.asarray(inputs["bn_beta"], f32)
    b2 = np.asarray(inputs["mlp_b2"], f32)  # [2]

    iota_row = np.tile(np.arange(P, dtype=f32), (P, 1))
    identity = np.eye(P, dtype=f32)
    ones_row = np.ones((1, P), f32)
    ones_col = np.ones((P, 1), f32)
    zeros_pool = np.zeros((P, 4 * 129), f32)

    in_maps = []
    for c in range(NCORES):
        ids = plan.order[c]
        sl = plan.slot_of[ids]
        zidx = np.zeros(S_LOCAL, np.int64)
        nidx = np.zeros(S_LOCAL, np.int64)
        zidx[sl] = z[ids]
        nidx[sl] = node_id[ids] % EMB_CHUNK
        ez = np.zeros((P, S_LOCAL // 16), np.int16)
        en = np.zeros((P, S_LOCAL // 16), np.int16)
        _wrap_idx_into(ez, 0, zidx.astype(np.int16))
        _wrap_idx_into(en, 0, nidx.astype(np.int16))
        bs4 = np.full((S_LOCAL, 4), -1.0, f32)
        gids = batch[ids]
        q = gids // P
        bs4[sl, q] = (gids - q * P).astype(f32)
        bs4 = np.ascontiguousarray(
            bs4.reshape(NT, P, 4).transpose(1, 0, 2).reshape(P, NT * 4)
        )
        eidx, dstslot = plan.core_tables(c)
        in_maps.append(
            {
                "z_emb": z_emb, "node_emb": node_emb,
                "eidx": eidx, "ez": ez, "en": en,
                "dstslot": dstslot, "bs4": bs4,
                "iota_row": iota_row, "identity": identity,
                "ones_row": ones_row, "ones_col": ones_col,
                "zeros_pool": zeros_pool,
                "brow": brow, "W": Ws,
                "mlp_W1": mlp_W1, "mlp_W2t": np.ascontiguousarray(mlp_W2),
                "gam": gam.reshape(P, 1), "tprime": tprime.reshape(P, 1),
                "b2c": b2.reshape(2, 1),
            }
        )

    # ---------------- build the bass program ----------------
    nc = bacc.Bacc(dynamic_dma_scratch_size=32768)
    dp = lambda n, s, d: nc.declare_dram_parameter(n, s, d, isOutput=False)
    d_zemb = dp("z_emb", [1000, P], dt.float32)
    d_nemb = dp("node_emb", [NODE_VOCAB, P], dt.float32)
    d_eidx = dp("eidx", [P, plan.tot_slots // 16], dt.int16)
    d_ez = dp("ez", [P, S_LOCAL // 16], dt.int16)
    d_en = dp("en", [P, S_LOCAL // 16], dt.int16)
    d_dst = dp("dstslot", [P, plan.nchunks], dt.float32)
    d_bs4 = dp("bs4", [P, NT * 4], dt.float32)
    d_iota = dp("iota_row", [P, P], dt.float32)
    d_ident = dp("identity", [P, P], dt.float32)
    d_ones = dp("ones_row", [1, P], dt.float32)
    d_onesc = dp("ones_col", [P, 1], dt.float32)
    d_zpool = dp("zeros_pool", [P, 4 * 129], dt.float32)
    d_brow = dp("brow", [3, 1, P], dt.float32)
    d_W = dp("W", [4, P, P], dt.float32)
    d_mW1 = dp("mlp_W1", [P, P], dt.float32)
    d_mW2 = dp("mlp_W2t", [P, 2], dt.float32)
    d_gam = dp("gam", [P, 1], dt.float32)
    d_tpr = dp("tprime", [P, 1], dt.float32)
    d_b2c = dp("b2c", [2, 1], dt.float32)
    d_out = nc.declare_dram_parameter("out", [2, N_GRAPHS], dt.float32, isOutput=True)

    y_loc = [nc.dram_tensor(f"y{l}_loc", [S_LOCAL, P], dt.float32) for l in range(3)]
    y_full = [
        nc.dram_tensor(
            f"y{l}_full", [NCORES * S_LOCAL, P], dt.float32, addr_space="Shared"
        )
        for l in range(3)
    ]
    pool_loc = nc.dram_tensor("pool_loc", [P, 4 * 129], dt.float32)
    pool_sum = nc.dram_tensor("pool_sum", [P, 4 * 129], dt.float32, addr_space="Shared")

    TWO_S = 2 * S_LOCAL
    rg = [list(range(NCORES))]

    with TileContext(nc) as tc:
        with (
            tc.tile_pool(name="const", bufs=1) as cpool,
            tc.tile_pool(name="gbuf", bufs=3) as gpool,
            tc.tile_pool(name="work", bufs=3) as wpool,
            tc.tile_pool(name="spool", bufs=4) as spool,
            tc.tile_pool(name="psAgg", bufs=1, space="PSUM") as psA,
            tc.tile_pool(name="psTr", bufs=2, space="PSUM") as psB,
            tc.tile_pool(name="psMm", bufs=1, space="PSUM") as psC,
        ):
            # ---- resident constants / tables ----
            iota_t = cpool.tile([P, P], dt.float32)
            nc.sync.dma_start(out=iota_t[:], in_=d_iota[:])
            ident_t = cpool.tile([P, P], dt.float32)
            nc.sync.dma_start(out=ident_t[:], in_=d_ident[:])
            ones_t = cpool.tile([1, P], dt.float32)
            nc.sync.dma_start(out=ones_t[:], in_=d_ones[:])
            onesc_t = cpool.tile([P, 1], dt.float32)
            nc.sync.dma_start(out=onesc_t[:], in_=d_onesc[:])
            brow_t = cpool.tile([1, 3, P], dt.float32)
            nc.sync.dma_start(out=brow_t[:], in_=d_brow.rearrange("b o f -> o b f"))
            W_t = cpool.tile([P, 4, P], dt.float32)
            nc.sync.dma_start(out=W_t[:], in_=d_W.rearrange("w k f -> k w f"))
            dst_t = cpool.tile([P, plan.nchunks], dt.float32)
            nc.sync.dma_start(out=dst_t[:], in_=d_dst[:])
            bs4_t = cpool.tile([P, NT * 4], dt.float32)
            nc.sync.dma_start(out=bs4_t[:], in_=d_bs4[:])
            pool_acc = cpool.tile([P, 4 * 129], dt.float32)
            nc.sync.dma_start(out=pool_acc[:], in_=d_zpool[:])

            # ---- layer 0: embeddings -> y0 rows ----
            with tc.tile_pool(name="l0", bufs=1) as l0pool:
                ez_t = l0pool.tile([P, S_LOCAL // 16], dt.int16)
                nc.sync.dma_start(out=ez_t[:], in_=d_ez[:])
                en_t = l0pool.tile([P, S_LOCAL // 16], dt.int16)
                nc.sync.dma_start(out=en_t[:], in_=d_en[:])
                zx = l0pool.tile([P, NT, P], dt.float32)
                nc.gpsimd.dma_gather(
                    out_ap=zx[:], in_ap=d_zemb[:], idxs_ap=ez_t[:],
                    num_idxs=S_LOCAL, num_idxs_reg=S_LOCAL, elem_size=P,
                    single_packet=False,
                )
                for k in range(7):
                    cap = int(plan.emb_caps[k])
                    if cap == 0:
                        continue
                    off = int(plan.emb_offs[k])
                    nxc = gpool.tile([P, cap // P, P], dt.float32, tag="nx")
                    base = k * EMB_CHUNK
                    nrows = min(EMB_CHUNK, NODE_VOCAB - base)
                    nc.gpsimd.dma_gather(
                        out_ap=nxc[:], in_ap=d_nemb[base : base + nrows, :],
                        idxs_ap=en_t[:, off // 16 : (off + cap) // 16],
                        num_idxs=cap, num_idxs_reg=cap, elem_size=P,
                        single_packet=False,
                    )
                    for j in range(cap // P):
                        t = off // P + j
                        zt_ps = psB.tile([P, P], dt.float32, tag="tr")
                        nc.tensor.transpose(zt_ps[:], zx[:, t, :], ident_t[:])
                        zT = spool.tile([P, P], dt.float32, tag="zT")
                        nc.vector.tensor_copy(out=zT[:], in_=zt_ps[:])
                        nt_ps = psB.tile([P, P], dt.float32, tag="tr")
                        nc.tensor.transpose(nt_ps[:], nxc[:, j, :], ident_t[:])
                        nT = spool.tile([P, P], dt.float32, tag="nT")
                        nc.vector.tensor_copy(out=nT[:], in_=nt_ps[:])
                        y_ps = psC.tile([P, P], dt.float32, tag="mm2")
                        nc.tensor.matmul(y_ps[:], zT[:], W_t[:, 0, :], start=True, stop=False)
                        nc.tensor.matmul(y_ps[:], nT[:], W_t[:, 1, :], start=False, stop=True)
                        yrow = wpool.tile([P, P], dt.float32, tag="yrow")
                        nc.vector.tensor_copy(out=yrow[:], in_=y_ps[:])
                        nc.sync.dma_start(
                            out=y_loc[0][t * P : (t + 1) * P, :], in_=yrow[:]
                        )

            nc.gpsimd.collective_compute(
                "AllGather", mybir.AluOpType.bypass, replica_groups=rg,
                ins=[y_loc[0][:]], outs=[y_full[0][:]],
            )

            # ---- conv stages ----
            for stage in range(3):
                relu = stage < 2
                yf = y_full[stage]
                for gi, g in enumerate(plan.groups):
                    agg_ps = {
                        t: psA.tile(
                            [P, P], dt.float32, tag=f"agg{t % GROUP_T}",
                            name=f"agg_{stage}_{t}",
                        )
                        for t in g
                    }
                    started = {t: False for t in g}
                    for c4 in range(4):
                        off, ln = plan.instr[(gi, c4)]
                        if ln == 0:
                            continue
                        G = gpool.tile([P, ln // P, P], dt.float32, tag="G")
                        et = gpool.tile([P, ln // 16], dt.int16, tag="et")
                        nc.sync.dma_start(
                            out=et[:], in_=d_eidx[:, off // 16 : (off + ln) // 16]
                        )
                        nc.gpsimd.dma_gather(
                            out_ap=G[:],
                            in_ap=yf[c4 * TWO_S : (c4 + 1) * TWO_S, :],
                            idxs_ap=et[:],
                            num_idxs=ln, num_idxs_reg=ln, elem_size=P,
                            single_packet=False,
                        )
                        col = 0
                        for t in g:
                            ch0 = plan.chunk_of_seg[(t, c4)]
                            for j in range(int(plan.seg_caps[t, c4]) // P):
                                S = spool.tile([P, P], dt.float32, tag="S")
                                nc.vector.tensor_scalar(
                                    out=S[:], in0=iota_t[:],
                                    scalar1=dst_t[:, ch0 + j : ch0 + j + 1],
                                    scalar2=None, op0=mybir.AluOpType.is_equal,
                                )
                                nc.tensor.matmul(
                                    agg_ps[t][:], S[:], G[:, col, :],
                                    start=not started[t], stop=False,
                                )
                                started[t] = True
                                col += 1
                    for t in g:
                        nc.tensor.matmul(
                            agg_ps[t][:], ones_t[:], brow_t[:, stage, :],
                            start=not started[t], stop=True,
                        )
                        xrow = wpool.tile([P, P], dt.float32, tag="xrow")
                        if relu:
                            nc.vector.tensor_scalar(
                                out=xrow[:], in0=agg_ps[t][:], scalar1=0.0,
                                scalar2=None, op0=mybir.AluOpType.max,
                            )
                        else:
                            nc.vector.tensor_copy(out=xrow[:], in_=agg_ps[t][:])
                        if stage < 2:
                            tr_ps = psB.tile([P, P], dt.float32, tag="tr")
                            nc.tensor.transpose(tr_ps[:], xrow[:], ident_t[:])
                            xT = spool.tile([P, P], dt.float32, tag="xT")
                            nc.vector.tensor_copy(out=xT[:], in_=tr_ps[:])
                            y_ps = psC.tile([P, P], dt.float32, tag="mm2")
                            nc.tensor.matmul(
                                y_ps[:], xT[:], W_t[:, stage + 2, :],
                                start=True, stop=True,
                            )
                            yrow = wpool.tile([P, P], dt.float32, tag="yrow")
                            nc.vector.tensor_copy(out=yrow[:], in_=y_ps[:])
                            nc.sync.dma_start(
                                out=y_loc[stage + 1][t * P : (t + 1) * P, :],
                                in_=yrow[:],
                            )
                        else:
                            x3e = wpool.tile([P, 129], dt.float32, tag="x3e")
                            nc.vector.tensor_copy(out=x3e[:, :P], in_=xrow[:])
                            nc.vector.tensor_copy(out=x3e[:, P : P + 1], in_=onesc_t[:])
                            for q in range(4):
                                M = spool.tile([P, P], dt.float32, tag="M")
                                nc.vector.tensor_scalar(
                                    out=M[:], in0=iota_t[:],
                                    scalar1=bs4_t[:, t * 4 + q : t * 4 + q + 1],
                                    scalar2=None, op0=mybir.AluOpType.is_equal,
                                )
                                pm_ps = psC.tile([P, 129], dt.float32, tag="mm2")
                                nc.tensor.matmul(
                                    pm_ps[:], M[:], x3e[:], start=True, stop=True
                                )
                                nc.vector.tensor_tensor(
                                    out=pool_acc[:, q * 129 : (q + 1) * 129],
                                    in0=pool_acc[:, q * 129 : (q + 1) * 129],
                                    in1=pm_ps[:], op=mybir.AluOpType.add,
                                )
                if stage < 2:
                    nc.gpsimd.collective_compute(
                        "AllGather", mybir.AluOpType.bypass, replica_groups=rg,
                        ins=[y_loc[stage + 1][:]], outs=[y_full[stage + 1][:]],
                    )

            # ---- pooled partial sums -> AllReduce -> MLP head ----
            nc.sync.dma_start(out=pool_loc[:], in_=pool_acc[:])
            nc.gpsimd.collective_compute(
                "AllReduce", mybir.AluOpType.add, replica_groups=rg,
                ins=[pool_loc[:]], outs=[pool_sum[:]],
            )
            mW1_t = cpool.tile([P, P], dt.float32)
            nc.sync.dma_start(out=mW1_t[:], in_=d_mW1[:])
            mW2_t = cpool.tile([P, 2], dt.float32)
            nc.sync.dma_start(out=mW2_t[:], in_=d_mW2[:])
            gam_t = cpool.tile([P, 1], dt.float32)
            nc.sync.dma_start(out=gam_t[:], in_=d_gam[:])
            tpr_t = cpool.tile([P, 1], dt.float32)
            nc.sync.dma_start(out=tpr_t[:], in_=d_tpr[:])
            b2_t = cpool.tile([2, 1], dt.float32)
            nc.sync.dma_start(out=b2_t[:], in_=d_b2c[:])

            psum_all = cpool.tile([P, 4 * 129], dt.float32)
            nc.sync.dma_start(out=psum_all[:], in_=pool_sum[:])
            meanT = cpool.tile([P, 4 * P], dt.float32)
            for q in range(4):
                cnt = wpool.tile([P, 1], dt.float32, tag="cnt")
                nc.vector.tensor_scalar(
                    out=cnt[:], in0=psum_all[:, q * 129 + P : q * 129 + P + 1],
                    scalar1=1.0, scalar2=None, op0=mybir.AluOpType.max,
                )
                rec = wpool.tile([P, 1], dt.float32, tag="rec")
                nc.vector.reciprocal(out=rec[:], in_=cnt[:])
                mean = wpool.tile([P, P], dt.float32, tag="mean")
                nc.vector.tensor_scalar(
                    out=mean[:], in0=psum_all[:, q * 129 : q * 129 + P],
                    scalar1=rec[:], scalar2=None, op0=mybir.AluOpType.mult,
                )
                mt_ps = psB.tile([P, P], dt.float32, tag="tr")
                nc.tensor.transpose(mt_ps[:], mean[:], ident_t[:])
                nc.vector.tensor_copy(out=meanT[:, q * P : (q + 1) * P], in_=mt_ps[:])
            h_ps = psC.tile([P, 4 * P], dt.float32, tag="hps")
            nc.tensor.matmul(h_ps[:], mW1_t[:], meanT[:], start=True, stop=True)
            h2 = cpool.tile([P, 4 * P], dt.float32)
            nc.scalar.activation(
                out=h2[:], in_=h_ps[:], func=mybir.ActivationFunctionType.Relu,
                bias=tpr_t[:], scale=gam_t[:],
            )
            o_ps = psB.tile([2, 4 * P], dt.float32, tag="tr")
            nc.tensor.matmul(o_ps[:], mW2_t[:], h2[:], start=True, stop=True)
            osb = cpool.tile([2, 4 * P], dt.float32)
            nc.vector.tensor_scalar(
                out=osb[:], in0=o_ps[:], scalar1=b2_t[:], scalar2=None,
                op0=mybir.AluOpType.add,
            )
            nc.sync.dma_start(out=d_out[:], in_=osb[:])

    nc.finalize()
    return nc, in_maps


def kernel(**inputs) -> np.ndarray:
    from concourse.bass_utils import run_bass_kernel_spmd

    nc, in_maps = _build(inputs)
    res = run_bass_kernel_spmd(nc, in_maps, list(range(NCORES)))
    return np.ascontiguousarray(np.asarray(res.results[0]["out"]).T.astype(np.float32))
